# revision 22
# baseline (speedup 1.0000x reference)
"""Trainium2 Bass kernel for DetectionLayer (refine + per-class NMS).

Contract: kernel(rois, probs, deltas) with FULL inputs
  rois   [16, 4096, 4]   f32
  probs  [16, 4096, 81]  f32
  deltas [16, 4096, 81, 4] f32
returns [16, 100, 6] f32 detections, matching the jax reference.

Sharding: pure data parallel - 2 images per core across 8 NeuronCores.

Fast path (always): DMA both images' probs, count elements >= 0.7 with a
DVE is_ge+accum / ACT sign+accum split, sum via PE ones-matmul.  The
zeroed output is DMA'd to HBM up front.
Guard (tc.If, only when count > 0): deltas load, per-argmax-class box
refine, and a fixed 100-iteration per-class NMS per image, then the real
detections overwrite the zeros in HBM.
"""

import os as _os

import numpy as np

import concourse.bacc as bacc
import concourse.bass as bass
import concourse.bass_isa as bass_isa
import concourse.mybir as mybir
from concourse.tile import TileContext

B = 16              # full batch
NCORES = 8
BPC = B // NCORES   # images per core
N = 4096            # rois per image
C = 81              # classes
K = 100             # detection_max_instances
P = 128             # SBUF partitions
NP = N // P         # rois per partition per image (32)
NEG = -1e9
MIN_CONF = 0.7
NMS_T = 0.3
F32 = mybir.dt.float32
I32 = mybir.dt.int32

# gate split: DVE handles rois [0, NA), ACT handles [NA, NP) of each image
DEBUG = _os.environ.get("DETK_DEBUG", "0") == "1"
NOGUARD = _os.environ.get("DETK_NOGUARD", "0") == "1"
# probs DMA chunks in issue order: (img, lo, hi, dve_rois, queue)
# queue 0 = SP HWDGE (rings prioritize it), 1 = Act HWDGE (lands last).
# Per chunk, DVE counts rois [lo, lo+dve) via is_ge+sum (coeff 2) and the
# ACT engine does [lo+dve, hi) via sign+sum (coeff 1). Chunks are sized so
# the last-landing chunks carry little gate work.
CHUNKS = [
    (0, 0, 32, 17, 0),
    (1, 0, 18, 13, 0),
    (1, 18, 26, 8, 0),
    (1, 26, 32, 0, 1),
]
_DVE_N = [d for (_, _, _, d, _) in CHUNKS if d > 0]
_ACT_N = [hi - lo - d for (_, lo, hi, d, _) in CHUNKS if hi - lo - d > 0]
# total elements processed by ACT (sign-sum) chunks
TOTAL_B_ELEMS = float(P * C * sum(_ACT_N))


def _refine_image(nc, tc, sm, img, ptw, scw, rt, dt_, crev, state):
    """Cold path per image: select argmax-class delta, refine boxes, build
    NMS state. All tiles are [..] slices of twin tensors at free index img."""
    pt = ptw[:, img]          # [P, NP, C]
    scores = scw[:, img]      # [P, NP]

    nc.vector.reduce_max(scores, pt, axis=mybir.AxisListType.X)
    ge = sm.tile([P, NP], F32, tag=f"ge{img}")
    nc.vector.tensor_single_scalar(ge, scores, MIN_CONF,
                                   op=mybir.AluOpType.is_ge)

    # one-hot mask of argmax class: M = (probs == score), in place over probs
    m = pt
    nc.vector.tensor_tensor(
        m, pt, scores.unsqueeze(2).to_broadcast([P, NP, C]),
        op=mybir.AluOpType.is_equal,
    )

    # select argmax-class delta: deltas *= M (bcast over k), sum over c
    d_perm = dt_.rearrange("p n c k -> p n k c")
    nc.vector.tensor_tensor(
        d_perm, d_perm, m.unsqueeze(2).to_broadcast([P, NP, 4, C]),
        op=mybir.AluOpType.mult,
    )
    dsel = sm.tile([P, NP, 4], F32, tag=f"dsel{img}")
    nc.vector.reduce_sum(dsel, d_perm, axis=mybir.AxisListType.X)

    # class id = 80 - max((80-c) * M)  (ties -> smallest c, like argmax)
    nc.vector.tensor_tensor(m, m, crev, op=mybir.AluOpType.mult)
    cid = sm.tile([P, NP], F32, tag=f"cid{img}")
    nc.vector.reduce_max(cid, m, axis=mybir.AxisListType.X)
    nc.vector.tensor_scalar(
        out=cid, in0=cid, scalar1=-1.0, scalar2=float(C - 1),
        op0=mybir.AluOpType.mult, op1=mybir.AluOpType.add,
    )

    # bbox_std scaling (match reference op order exactly)
    nc.vector.tensor_scalar_mul(dsel[:, :, 0:2], dsel[:, :, 0:2], 0.1)
    nc.vector.tensor_scalar_mul(dsel[:, :, 2:4], dsel[:, :, 2:4], 0.2)

    # ---- apply deltas + clip (mirrors _apply_deltas fp32 op order) ----
    h = sm.tile([P, NP], F32, tag=f"h{img}")
    w = sm.tile([P, NP], F32, tag=f"w{img}")
    nc.vector.tensor_sub(h, rt[:, :, 2], rt[:, :, 0])
    nc.vector.tensor_sub(w, rt[:, :, 3], rt[:, :, 1])
    t1 = sm.tile([P, NP], F32, tag=f"t1{img}")
    t2 = sm.tile([P, NP], F32, tag=f"t2{img}")
    cy = sm.tile([P, NP], F32, tag=f"cy{img}")
    cx = sm.tile([P, NP], F32, tag=f"cx{img}")
    nc.vector.tensor_scalar_mul(t1, h, 0.5)
    nc.vector.tensor_add(t2, rt[:, :, 0], t1)
    nc.vector.tensor_mul(t1, dsel[:, :, 0], h)
    nc.vector.tensor_add(cy, t2, t1)
    nc.vector.tensor_scalar_mul(t1, w, 0.5)
    nc.vector.tensor_add(t2, rt[:, :, 1], t1)
    nc.vector.tensor_mul(t1, dsel[:, :, 1], w)
    nc.vector.tensor_add(cx, t2, t1)
    e = sm.tile([P, NP], F32, tag=f"e{img}")
    nc.scalar.activation(e, dsel[:, :, 2], mybir.ActivationFunctionType.Exp)
    nc.vector.tensor_mul(h, h, e)
    nc.scalar.activation(e, dsel[:, :, 3], mybir.ActivationFunctionType.Exp)
    nc.vector.tensor_mul(w, w, e)

    ref = sm.tile([P, NP, 4], F32, tag=f"ref{img}")
    nc.vector.tensor_scalar_mul(t1, h, 0.5)
    nc.vector.tensor_sub(ref[:, :, 0], cy, t1)
    nc.vector.tensor_add(ref[:, :, 2], cy, t1)
    nc.vector.tensor_scalar_mul(t2, w, 0.5)
    nc.vector.tensor_sub(ref[:, :, 1], cx, t2)
    nc.vector.tensor_add(ref[:, :, 3], cx, t2)
    nc.vector.tensor_scalar(
        out=ref, in0=ref, scalar1=0.0, scalar2=1.0,
        op0=mybir.AluOpType.max, op1=mybir.AluOpType.min,
    )

    # ---- NMS state ----
    sc = state["sc"][:, img]
    ob = state["ob"][:, img]
    ar = state["ar"][:, img]
    cat = state["cat"][:, img]
    negs = state["negs"]

    vf = sm.tile([P, NP], F32, tag=f"vf{img}")
    nc.vector.tensor_single_scalar(vf, cid, 0.5, op=mybir.AluOpType.is_ge)
    v = sm.tile([P, NP], mybir.dt.uint8, tag=f"v{img}")
    nc.vector.tensor_mul(v, vf, ge)
    nc.vector.tensor_copy(sc, negs)
    nc.vector.copy_predicated(sc, v, scores)

    nc.vector.scalar_tensor_tensor(
        out=ob, in0=cid.unsqueeze(2).to_broadcast([P, NP, 4]), scalar=2.0,
        in1=ref, op0=mybir.AluOpType.mult, op1=mybir.AluOpType.add,
    )
    ar2 = sm.tile([P, NP, 2], F32, tag=f"ar2{img}")
    nc.vector.tensor_sub(ar2, ob[:, :, 2:4], ob[:, :, 0:2])
    nc.vector.tensor_mul(ar, ar2[:, :, 0], ar2[:, :, 1])
    nc.vector.tensor_copy(cat[:, :, 0:4], ref)
    nc.vector.tensor_copy(cat[:, :, 4], cid)
    nc.vector.tensor_copy(cat[:, :, 5], scores)


def _nms_image(nc, tc, sm, img, det, state):
    """Cold path per image: fixed K-iteration NMS; rows past exhaustion are
    written as exact zeros (gm == NEG gate)."""
    sc = state["sc"][:, img]
    ob = state["ob"][:, img]
    ar = state["ar"][:, img]
    cat = state["cat"][:, img]
    negs = state["negs"]
    mr = state["mr"]

    with tc.For_i(0, K, name=f"nms{img}") as i:
        pm = sm.tile([P, 1], F32, tag=f"pm{img}")
        nc.vector.reduce_max(pm, sc, axis=mybir.AxisListType.X)
        gm = sm.tile([P, 1], F32, tag=f"gm{img}")
        nc.gpsimd.partition_all_reduce(gm, pm, channels=P,
                                       reduce_op=bass_isa.ReduceOp.max)
        msk = sm.tile([P, NP], F32, tag=f"msk{img}")
        nc.vector.tensor_tensor(msk, sc, gm.to_broadcast([P, NP]),
                                op=mybir.AluOpType.is_equal)
        mb6 = sm.tile([P, NP, 6], F32, tag=f"mb6{img}")
        nc.vector.tensor_tensor(
            mb6, cat, msk.unsqueeze(2).to_broadcast([P, NP, 6]),
            op=mybir.AluOpType.mult,
        )
        r6p = sm.tile([P, 6], F32, tag=f"r6p{img}")
        nc.vector.reduce_sum(r6p, mb6.rearrange("p n k -> p k n"),
                             axis=mybir.AxisListType.X)
        r6 = sm.tile([P, 6], F32, tag=f"r6{img}")
        nc.gpsimd.partition_all_reduce(r6, r6p, channels=P,
                                       reduce_op=bass_isa.ReduceOp.add)
        okm = sm.tile([P, 1], F32, tag=f"okm{img}")
        nc.vector.tensor_single_scalar(okm, gm, NEG * 0.5,
                                       op=mybir.AluOpType.is_gt)
        nc.vector.tensor_mul(r6, r6, okm.to_broadcast([P, 6]))
        nc.vector.tensor_copy(det[img][0:1, bass.ds(i * 6, 6)],
                              r6[0:1, :])

        sb = sm.tile([P, 4], F32, tag=f"sb{img}")
        nc.vector.scalar_tensor_tensor(
            out=sb, in0=r6[:, 4:5].to_broadcast([P, 4]), scalar=2.0,
            in1=r6[:, 0:4], op0=mybir.AluOpType.mult, op1=mybir.AluOpType.add,
        )
        mx = sm.tile([P, NP, 2], F32, tag=f"mx{img}")
        nc.vector.tensor_tensor(
            mx, ob[:, :, 0:2], sb[:, 0:2].unsqueeze(1).to_broadcast([P, NP, 2]),
            op=mybir.AluOpType.max,
        )
        mn = sm.tile([P, NP, 2], F32, tag=f"mn{img}")
        nc.vector.tensor_tensor(
            mn, ob[:, :, 2:4], sb[:, 2:4].unsqueeze(1).to_broadcast([P, NP, 2]),
            op=mybir.AluOpType.min,
        )
        nc.vector.tensor_sub(mn, mn, mx)
        nc.vector.tensor_scalar_max(mn, mn, 0.0)
        inter = sm.tile([P, NP], F32, tag=f"inter{img}")
        nc.vector.tensor_mul(inter, mn[:, :, 0], mn[:, :, 1])
        aa2 = sm.tile([P, 2], F32, tag=f"aa2{img}")
        nc.vector.tensor_sub(aa2, sb[:, 2:4], sb[:, 0:2])
        aa = sm.tile([P, 1], F32, tag=f"aa{img}")
        nc.vector.tensor_mul(aa, aa2[:, 0:1], aa2[:, 1:2])
        u = sm.tile([P, NP], F32, tag=f"u{img}")
        nc.vector.scalar_tensor_tensor(
            out=u, in0=ar, scalar=aa[:, 0:1], in1=inter,
            op0=mybir.AluOpType.add, op1=mybir.AluOpType.subtract,
        )
        sup = sm.tile([P, NP], mybir.dt.uint8, tag=f"sup{img}")
        nc.vector.scalar_tensor_tensor(
            out=sup, in0=u, scalar=NMS_T, in1=inter,
            op0=mybir.AluOpType.mult, op1=mybir.AluOpType.is_lt,
        )
        nc.vector.copy_predicated(sc, sup, negs)
        nc.vector.tensor_copy(mr[:, 0:1], gm)
        nc.vector.match_replace(out=sc, in_to_replace=mr, in_values=sc,
                                imm_value=NEG)


def build_nc():
    nc = bacc.Bacc("TRN2", target_bir_lowering=False)
    rois_t = nc.dram_tensor("rois", [BPC, N, 4], F32, kind="ExternalInput")
    probs_t = nc.dram_tensor("probs", [BPC, N, C], F32, kind="ExternalInput")
    deltas_t = nc.dram_tensor("deltas", [BPC, N, C, 4], F32, kind="ExternalInput")
    out_t = nc.dram_tensor("out", [BPC, K, 6], F32, kind="ExternalOutput")
    dbg_t = None
    if DEBUG:
        dbg_t = nc.dram_tensor("dbg", [1, 16], F32, kind="ExternalOutput")

    with TileContext(nc) as tc:
        with (
            tc.tile_pool(name="big", bufs=1) as big,
            tc.tile_pool(name="small", bufs=1) as sm,
            tc.tile_pool(name="psum", bufs=1, space="PSUM") as pp,
        ):
            # ---------------- fast path ----------------
            # probs for both images in one twin tile, DMA'd per CHUNKS on
            # two HWDGE queues (SP + Act) for parallel descriptor gen
            ptw = big.tile([P, BPC, NP, C], F32, tag="probs")
            psrc = [probs_t[b].rearrange("(p n) c -> p n c", p=P)
                    for b in range(BPC)]
            for b, lo, hi, _, q in CHUNKS:
                eng = nc.sync if q == 0 else nc.scalar
                eng.dma_start(out=ptw[:, b, lo:hi], in_=psrc[b][:, lo:hi])

            det0 = sm.tile([1, K * 6], F32, tag="det0")
            det1 = sm.tile([1, K * 6], F32, tag="det1")
            det = [det0, det1]
            nc.vector.memset(det0, 0.0)
            nc.gpsimd.memset(det1, 0.0)

            # zeros out-DMA up front; real detections overwrite in the guard
            out_aps = []
            for img in range(BPC):
                ap = out_t[img].rearrange("k s -> (k s)").unsqueeze(0)
                out_aps.append(ap)
                nc.sync.dma_start(out=ap, in_=det[img][0:1])

            # element count >= MIN_CONF: DVE is_ge+sum (coeff 2) then
            # ACT sign+sum (coeff 1); cnt cols = DVE chunks then ACT chunks
            NDVE = len(_DVE_N)
            NCOL = NDVE + len(_ACT_N) + 1
            cnt = sm.tile([P, NCOL], F32, tag="cnt")
            # last col pre-set to the per-partition sign-sum offset so the
            # final combine is just matmul + reduce
            nc.vector.memset(cnt[:, NCOL - 1:NCOL],
                             float(C * sum(_ACT_N)))
            scrA = sm.tile([P, max(_DVE_N), C], mybir.dt.uint8, tag="scrA")
            scrB = sm.tile([P, max(_ACT_N), C], mybir.dt.bfloat16,
                           tag="scrB")
            biasT = sm.tile([P, 1], F32, tag="biasT")
            nc.gpsimd.memset(biasT, -MIN_CONF)

            col_dve, col_act = 0, NDVE
            for b, lo, hi, dve_n, _ in CHUNKS:
                if dve_n > 0:
                    nc.vector.tensor_scalar(
                        out=scrA[:, 0:dve_n], in0=ptw[:, b, lo:lo + dve_n],
                        scalar1=MIN_CONF, scalar2=None,
                        op0=mybir.AluOpType.is_ge, op1=mybir.AluOpType.add,
                        accum_out=cnt[:, col_dve:col_dve + 1],
                    )
                    col_dve += 1
                act_n = hi - lo - dve_n
                if act_n > 0:
                    nc.scalar.activation(
                        scrB[:, 0:act_n], ptw[:, b, lo + dve_n:hi],
                        mybir.ActivationFunctionType.Sign,
                        bias=biasT[:, 0:1],
                        accum_out=cnt[:, col_act:col_act + 1],
                    )
                    col_act += 1

            ones = sm.tile([P, 1], F32, tag="ones")
            nc.vector.memset(ones, 1.0)
            # g = 2*sum(DVE counts) + sum(ACT sign sums) + #ACT-elems
            #   = 2 * (total elements >= MIN_CONF)   (exact in f32)
            nc.vector.tensor_scalar_mul(cnt[:, 0:NDVE], cnt[:, 0:NDVE], 2.0)
            csum = pp.tile([1, NCOL], F32, tag="csum")
            nc.tensor.matmul(csum, ones, cnt, start=True, stop=True)
            gi = sm.tile([1, 1], I32, tag="gi")
            with nc.allow_low_precision(
                    reason="exact small-int sum, int32 output"):
                nc.vector.reduce_sum(gi, csum, axis=mybir.AxisListType.X)

            if DEBUG:
                dbgs = sm.tile([1, 8], F32, tag="dbgs")
                nc.vector.memset(dbgs, 0.0)
                nc.vector.tensor_copy(dbgs[0:1, 0:4], cs)
                nc.vector.tensor_copy(dbgs[0:1, 4:5], ga)
                nc.vector.tensor_copy(dbgs[0:1, 5:6], gb)
                nc.sync.dma_start(out=dbg_t[0:1, 0:8], in_=dbgs)

            gv = nc.values_load(gi[0:1, 0:1], min_val=0,
                                max_val=2 * BPC * N * C,
                                skip_runtime_bounds_check=True)

            # ---------------- guarded cold path ----------------
            if not NOGUARD:
              with tc.If(gv >= 1):
                crev = sm.tile([P, NP, C], F32, tag="crev")
                nc.gpsimd.iota(crev, pattern=[[0, NP], [-1, C]], base=C - 1,
                               channel_multiplier=0,
                               allow_small_or_imprecise_dtypes=True)
                negs = sm.tile([P, NP], F32, tag="negs")
                nc.gpsimd.memset(negs, NEG)
                mr = sm.tile([P, 8], F32, tag="mr")
                nc.gpsimd.memset(mr, NEG)

                sc_w = sm.tile([P, BPC, NP], F32, tag="sc")
                ob_w = sm.tile([P, BPC, NP, 4], F32, tag="ob")
                ar_w = sm.tile([P, BPC, NP], F32, tag="ar")
                cat_w = sm.tile([P, BPC, NP, 6], F32, tag="cat")
                state = {
                    "negs": negs,
                    "mr": mr,
                    "sc": sc_w,
                    "ob": ob_w,
                    "ar": ar_w,
                    "cat": cat_w,
                }
                scw = sm.tile([P, BPC, NP], F32, tag="scores")

                for img in range(BPC):
                    rt = sm.tile([P, NP, 4], F32, tag=f"rois{img}")
                    nc.sync.dma_start(
                        out=rt,
                        in_=rois_t[img].rearrange("(p n) k -> p n k", p=P))
                    dt_ = big.tile([P, NP, C, 4], F32, tag=f"deltas{img}")
                    dsrc = deltas_t[img].rearrange("(p n) c k -> p n c k", p=P)
                    for s in range(8):
                        sl = slice(16 * s, 16 * s + 16)
                        nc.sync.dma_start(out=dt_[sl], in_=dsrc[sl])
                    _refine_image(nc, tc, sm, img, ptw, scw, rt, dt_, crev,
                                  state)
                if DEBUG:
                    pmd = sm.tile([P, 4], F32, tag="pmd")
                    nc.vector.reduce_max(pmd[:, 0:1], state["sc"][:, 0],
                                         axis=mybir.AxisListType.X)
                    nc.vector.reduce_max(pmd[:, 1:2], state["sc"][:, 1],
                                         axis=mybir.AxisListType.X)
                    nc.vector.reduce_max(pmd[:, 2:3], scw[:, 0],
                                         axis=mybir.AxisListType.X)
                    nc.vector.reduce_max(pmd[:, 3:4], scw[:, 1],
                                         axis=mybir.AxisListType.X)
                    pmg = sm.tile([P, 4], F32, tag="pmg")
                    nc.gpsimd.partition_all_reduce(
                        pmg, pmd, channels=P, reduce_op=bass_isa.ReduceOp.max)
                    nc.sync.dma_start(out=dbg_t[0:1, 8:12], in_=pmg[0:1, :])
                for img in range(BPC):
                    _nms_image(nc, tc, sm, img, det, state)
                if DEBUG:
                    dbgs2 = sm.tile([1, 4], F32, tag="dbgs2")
                    nc.vector.tensor_copy(dbgs2[0:1, 0:2], det[0][0:1, 0:2])
                    nc.vector.tensor_copy(dbgs2[0:1, 2:4], det[1][0:1, 0:2])
                    nc.sync.dma_start(out=dbg_t[0:1, 12:16], in_=dbgs2)
                for img in range(BPC):
                    fap = out_t[img].rearrange("k s -> (k s)").unsqueeze(0)
                    nc.sync.dma_start(out=fap, in_=det[img][0:1])
    nc.compile()
    return nc


LAST_RESULTS = None  # BassKernelResults of the most recent kernel() call


def kernel(rois, probs, deltas):
    global LAST_RESULTS
    from concourse import bass_utils

    nc = build_nc()
    in_maps = []
    for c in range(NCORES):
        sl = slice(c * BPC, (c + 1) * BPC)
        in_maps.append({
            "rois": np.ascontiguousarray(rois[sl], dtype=np.float32),
            "probs": np.ascontiguousarray(probs[sl], dtype=np.float32),
            "deltas": np.ascontiguousarray(deltas[sl], dtype=np.float32),
        })
    res = bass_utils.run_bass_kernel_spmd(nc, in_maps, core_ids=list(range(NCORES)))
    LAST_RESULTS = res
    return np.concatenate([r["out"] for r in res.results], axis=0)


if __name__ == "__main__":
    rng = np.random.default_rng(0)
    out = kernel(
        rng.random((B, N, 4), np.float32),
        rng.random((B, N, C), np.float32),
        rng.standard_normal((B, N, C, 4)).astype(np.float32),
    )
    print(out.shape, np.abs(out).max())


# revision 24
# speedup vs baseline: 1.1157x; 1.1157x over previous
"""Trainium2 Bass kernel for DetectionLayer (refine + per-class NMS).

Contract: kernel(rois, probs, deltas) with FULL inputs
  rois   [16, 4096, 4]   f32
  probs  [16, 4096, 81]  f32
  deltas [16, 4096, 81, 4] f32
returns [16, 100, 6] f32 detections, matching the jax reference.

Sharding: pure data parallel - 2 images per core across 8 NeuronCores.

Fast path (always): DMA both images' probs, count elements >= 0.7 with a
DVE is_ge+accum / ACT sign+accum split, sum via PE ones-matmul.  The
zeroed output is DMA'd to HBM up front.
Guard (tc.If, only when count > 0): deltas load, per-argmax-class box
refine, and a fixed 100-iteration per-class NMS per image, then the real
detections overwrite the zeros in HBM.
"""

import os as _os

import numpy as np

import concourse.bacc as bacc
import concourse.bass as bass
import concourse.bass_isa as bass_isa
import concourse.mybir as mybir
from concourse.tile import TileContext

B = 16              # full batch
NCORES = 8
BPC = B // NCORES   # images per core
N = 4096            # rois per image
C = 81              # classes
K = 100             # detection_max_instances
P = 128             # SBUF partitions
NP = N // P         # rois per partition per image (32)
NEG = -1e9
MIN_CONF = 0.7
NMS_T = 0.3
F32 = mybir.dt.float32
I32 = mybir.dt.int32

# gate split: DVE handles rois [0, NA), ACT handles [NA, NP) of each image
DEBUG = _os.environ.get("DETK_DEBUG", "0") == "1"
NOGUARD = _os.environ.get("DETK_NOGUARD", "0") == "1"
# probs DMA chunks in issue order: (img, lo, hi, dve_rois, queue)
# queue 0 = SP HWDGE (rings prioritize it), 1 = Act HWDGE (lands last).
# Per chunk, DVE counts rois [lo, lo+dve) via is_ge+sum (coeff 2) and the
# ACT engine does [lo+dve, hi) via sign+sum (coeff 1). Chunks are sized so
# the last-landing chunks carry little gate work.
CHUNKS = [
    (0, 0, 32, 17, 0),
    (1, 0, 18, 13, 0),
    (1, 18, 26, 8, 0),
    (1, 26, 32, 0, 1),
]
if _os.environ.get("DETK_CHUNKS"):
    import json as _json
    CHUNKS = [tuple(c) for c in _json.loads(_os.environ["DETK_CHUNKS"])]


def _chunk_meta(chunks):
    dve = [d for (_, _, _, d, _) in chunks if d > 0]
    act = [hi - lo - d for (_, lo, hi, d, _) in chunks if hi - lo - d > 0]
    return dve, act


_DVE_N, _ACT_N = _chunk_meta(CHUNKS)


def _refine_twin(nc, tc, sm, ptw, scw, rt, dt_, crev, state):
    """Cold path, both images at once: select argmax-class delta, refine
    boxes, build NMS state. Ops run on the flattened [P, BPC*NP] roi axis;
    the NMS loops later use per-image [:, img] slices of the same tiles."""
    NT = BPC * NP
    pt = ptw.rearrange("p b n c -> p (b n) c")          # [P, NT, C]
    scores = scw.rearrange("p b n -> p (b n)")          # [P, NT]
    rtf = rt.rearrange("p b n k -> p (b n) k")          # [P, NT, 4]

    nc.vector.reduce_max(scores, pt, axis=mybir.AxisListType.X)
    ge = sm.tile([P, NT], F32, tag="ge")
    nc.vector.tensor_single_scalar(ge, scores, MIN_CONF,
                                   op=mybir.AluOpType.is_ge)

    # one-hot mask of argmax class: M = (probs == score), in place over probs
    m = pt
    nc.vector.tensor_tensor(
        m, pt, scores.unsqueeze(2).to_broadcast([P, NT, C]),
        op=mybir.AluOpType.is_equal,
    )

    # select argmax-class delta: deltas *= M (bcast over k), sum over c
    d_perm = dt_.rearrange("p b n c k -> p (b n) k c")
    nc.vector.tensor_tensor(
        d_perm, d_perm, m.unsqueeze(2).to_broadcast([P, NT, 4, C]),
        op=mybir.AluOpType.mult,
    )
    dsel = sm.tile([P, NT, 4], F32, tag="dsel")
    nc.vector.reduce_sum(dsel, d_perm, axis=mybir.AxisListType.X)

    # class id = 80 - max((80-c) * M)  (ties -> smallest c, like argmax)
    nc.vector.tensor_tensor(m, m, crev, op=mybir.AluOpType.mult)
    cid = sm.tile([P, NT], F32, tag="cid")
    nc.vector.reduce_max(cid, m, axis=mybir.AxisListType.X)
    nc.vector.tensor_scalar(
        out=cid, in0=cid, scalar1=-1.0, scalar2=float(C - 1),
        op0=mybir.AluOpType.mult, op1=mybir.AluOpType.add,
    )

    # bbox_std scaling (match reference op order exactly)
    nc.vector.tensor_scalar_mul(dsel[:, :, 0:2], dsel[:, :, 0:2], 0.1)
    nc.vector.tensor_scalar_mul(dsel[:, :, 2:4], dsel[:, :, 2:4], 0.2)

    # ---- apply deltas + clip (mirrors _apply_deltas fp32 op order) ----
    h = sm.tile([P, NT], F32, tag="h")
    w = sm.tile([P, NT], F32, tag="w")
    nc.vector.tensor_sub(h, rtf[:, :, 2], rtf[:, :, 0])
    nc.vector.tensor_sub(w, rtf[:, :, 3], rtf[:, :, 1])
    t1 = sm.tile([P, NT], F32, tag="t1")
    t2 = sm.tile([P, NT], F32, tag="t2")
    cy = sm.tile([P, NT], F32, tag="cy")
    cx = sm.tile([P, NT], F32, tag="cx")
    nc.vector.tensor_scalar_mul(t1, h, 0.5)
    nc.vector.tensor_add(t2, rtf[:, :, 0], t1)
    nc.vector.tensor_mul(t1, dsel[:, :, 0], h)
    nc.vector.tensor_add(cy, t2, t1)
    nc.vector.tensor_scalar_mul(t1, w, 0.5)
    nc.vector.tensor_add(t2, rtf[:, :, 1], t1)
    nc.vector.tensor_mul(t1, dsel[:, :, 1], w)
    nc.vector.tensor_add(cx, t2, t1)
    e = sm.tile([P, NT], F32, tag="e")
    nc.scalar.activation(e, dsel[:, :, 2], mybir.ActivationFunctionType.Exp)
    nc.vector.tensor_mul(h, h, e)
    nc.scalar.activation(e, dsel[:, :, 3], mybir.ActivationFunctionType.Exp)
    nc.vector.tensor_mul(w, w, e)

    ref = sm.tile([P, NT, 4], F32, tag="ref")
    nc.vector.tensor_scalar_mul(t1, h, 0.5)
    nc.vector.tensor_sub(ref[:, :, 0], cy, t1)
    nc.vector.tensor_add(ref[:, :, 2], cy, t1)
    nc.vector.tensor_scalar_mul(t2, w, 0.5)
    nc.vector.tensor_sub(ref[:, :, 1], cx, t2)
    nc.vector.tensor_add(ref[:, :, 3], cx, t2)
    nc.vector.tensor_scalar(
        out=ref, in0=ref, scalar1=0.0, scalar2=1.0,
        op0=mybir.AluOpType.max, op1=mybir.AluOpType.min,
    )

    # ---- NMS state ----
    sc = state["sc"].rearrange("p b n -> p (b n)")
    ob = state["ob"].rearrange("p b n k -> p (b n) k")
    ar = state["ar"].rearrange("p b n -> p (b n)")
    cat = state["cat"].rearrange("p b n k -> p (b n) k")
    negs = state["negs"]

    vf = sm.tile([P, NT], F32, tag="vf")
    nc.vector.tensor_single_scalar(vf, cid, 0.5, op=mybir.AluOpType.is_ge)
    v = sm.tile([P, NT], mybir.dt.uint8, tag="v")
    nc.vector.tensor_mul(v, vf, ge)
    nc.vector.tensor_copy(sc, negs)
    nc.vector.copy_predicated(sc, v, scores)

    nc.vector.scalar_tensor_tensor(
        out=ob, in0=cid.unsqueeze(2).to_broadcast([P, NT, 4]), scalar=2.0,
        in1=ref, op0=mybir.AluOpType.mult, op1=mybir.AluOpType.add,
    )
    ar2 = sm.tile([P, NT, 2], F32, tag="ar2")
    nc.vector.tensor_sub(ar2, ob[:, :, 2:4], ob[:, :, 0:2])
    nc.vector.tensor_mul(ar, ar2[:, :, 0], ar2[:, :, 1])
    nc.vector.tensor_copy(cat[:, :, 0:4], ref)
    nc.vector.tensor_copy(cat[:, :, 4], cid)
    nc.vector.tensor_copy(cat[:, :, 5], scores)


def _nms_image(nc, tc, sm, img, det, state):
    """Cold path per image: fixed K-iteration NMS; rows past exhaustion are
    written as exact zeros (gm == NEG gate)."""
    sc = state["sc"][:, img]
    ob = state["ob"][:, img]
    ar = state["ar"][:, img]
    cat = state["cat"][:, img]
    negs = state["negs"][:, 0:NP]
    mr = state["mr"]

    with tc.For_i(0, K, name=f"nms{img}") as i:
        pm = sm.tile([P, 1], F32, tag=f"pm{img}")
        nc.vector.reduce_max(pm, sc, axis=mybir.AxisListType.X)
        gm = sm.tile([P, 1], F32, tag=f"gm{img}")
        nc.gpsimd.partition_all_reduce(gm, pm, channels=P,
                                       reduce_op=bass_isa.ReduceOp.max)
        msk = sm.tile([P, NP], F32, tag=f"msk{img}")
        nc.vector.tensor_tensor(msk, sc, gm.to_broadcast([P, NP]),
                                op=mybir.AluOpType.is_equal)
        mb6 = sm.tile([P, NP, 6], F32, tag=f"mb6{img}")
        nc.vector.tensor_tensor(
            mb6, cat, msk.unsqueeze(2).to_broadcast([P, NP, 6]),
            op=mybir.AluOpType.mult,
        )
        r6p = sm.tile([P, 6], F32, tag=f"r6p{img}")
        nc.vector.reduce_sum(r6p, mb6.rearrange("p n k -> p k n"),
                             axis=mybir.AxisListType.X)
        r6 = sm.tile([P, 6], F32, tag=f"r6{img}")
        nc.gpsimd.partition_all_reduce(r6, r6p, channels=P,
                                       reduce_op=bass_isa.ReduceOp.add)
        okm = sm.tile([P, 1], F32, tag=f"okm{img}")
        nc.vector.tensor_single_scalar(okm, gm, NEG * 0.5,
                                       op=mybir.AluOpType.is_gt)
        nc.vector.tensor_mul(r6, r6, okm.to_broadcast([P, 6]))
        nc.vector.tensor_copy(det[img][0:1, bass.ds(i * 6, 6)],
                              r6[0:1, :])

        sb = sm.tile([P, 4], F32, tag=f"sb{img}")
        nc.vector.scalar_tensor_tensor(
            out=sb, in0=r6[:, 4:5].to_broadcast([P, 4]), scalar=2.0,
            in1=r6[:, 0:4], op0=mybir.AluOpType.mult, op1=mybir.AluOpType.add,
        )
        mx = sm.tile([P, NP, 2], F32, tag=f"mx{img}")
        nc.vector.tensor_tensor(
            mx, ob[:, :, 0:2], sb[:, 0:2].unsqueeze(1).to_broadcast([P, NP, 2]),
            op=mybir.AluOpType.max,
        )
        mn = sm.tile([P, NP, 2], F32, tag=f"mn{img}")
        nc.vector.tensor_tensor(
            mn, ob[:, :, 2:4], sb[:, 2:4].unsqueeze(1).to_broadcast([P, NP, 2]),
            op=mybir.AluOpType.min,
        )
        nc.vector.tensor_sub(mn, mn, mx)
        nc.vector.tensor_scalar_max(mn, mn, 0.0)
        inter = sm.tile([P, NP], F32, tag=f"inter{img}")
        nc.vector.tensor_mul(inter, mn[:, :, 0], mn[:, :, 1])
        aa2 = sm.tile([P, 2], F32, tag=f"aa2{img}")
        nc.vector.tensor_sub(aa2, sb[:, 2:4], sb[:, 0:2])
        aa = sm.tile([P, 1], F32, tag=f"aa{img}")
        nc.vector.tensor_mul(aa, aa2[:, 0:1], aa2[:, 1:2])
        u = sm.tile([P, NP], F32, tag=f"u{img}")
        nc.vector.scalar_tensor_tensor(
            out=u, in0=ar, scalar=aa[:, 0:1], in1=inter,
            op0=mybir.AluOpType.add, op1=mybir.AluOpType.subtract,
        )
        sup = sm.tile([P, NP], mybir.dt.uint8, tag=f"sup{img}")
        nc.vector.scalar_tensor_tensor(
            out=sup, in0=u, scalar=NMS_T, in1=inter,
            op0=mybir.AluOpType.mult, op1=mybir.AluOpType.is_lt,
        )
        nc.vector.copy_predicated(sc, sup, negs)
        nc.vector.tensor_copy(mr[:, 0:1], gm)
        nc.vector.match_replace(out=sc, in_to_replace=mr, in_values=sc,
                                imm_value=NEG)


def build_nc():
    nc = bacc.Bacc("TRN2", target_bir_lowering=False)
    rois_t = nc.dram_tensor("rois", [BPC, N, 4], F32, kind="ExternalInput")
    probs_t = nc.dram_tensor("probs", [BPC, N, C], F32, kind="ExternalInput")
    deltas_t = nc.dram_tensor("deltas", [BPC, N, C, 4], F32, kind="ExternalInput")
    out_t = nc.dram_tensor("out", [BPC, K, 6], F32, kind="ExternalOutput")
    dbg_t = None
    if DEBUG:
        dbg_t = nc.dram_tensor("dbg", [1, 16], F32, kind="ExternalOutput")

    with TileContext(nc) as tc:
        with (
            tc.tile_pool(name="big", bufs=1) as big,
            tc.tile_pool(name="small", bufs=1) as sm,
            tc.tile_pool(name="psum", bufs=1, space="PSUM") as pp,
        ):
            # ---------------- fast path ----------------
            # probs for both images in one twin tile, DMA'd per CHUNKS on
            # two HWDGE queues (SP + Act) for parallel descriptor gen
            ptw = big.tile([P, BPC, NP, C], F32, tag="probs")
            psrc = [probs_t[b].rearrange("(p n) c -> p n c", p=P)
                    for b in range(BPC)]
            for b, lo, hi, _, q in CHUNKS:
                eng = nc.sync if q == 0 else nc.scalar
                eng.dma_start(out=ptw[:, b, lo:hi], in_=psrc[b][:, lo:hi])

            det0 = sm.tile([1, K * 6], F32, tag="det0")
            det1 = sm.tile([1, K * 6], F32, tag="det1")
            det = [det0, det1]
            nc.vector.memset(det0, 0.0)
            nc.gpsimd.memset(det1, 0.0)

            # zeros out-DMA up front; real detections overwrite in the guard
            out_aps = []
            for img in range(BPC):
                ap = out_t[img].rearrange("k s -> (k s)").unsqueeze(0)
                out_aps.append(ap)
                nc.sync.dma_start(out=ap, in_=det[img][0:1])

            # element count >= MIN_CONF: DVE is_ge+sum (coeff 2) then
            # ACT sign+sum (coeff 1); cnt cols = DVE chunks then ACT chunks
            NDVE = len(_DVE_N)
            NCOL = NDVE + len(_ACT_N) + 1
            cnt = sm.tile([P, NCOL], F32, tag="cnt")
            # last col pre-set to the per-partition sign-sum offset so the
            # final combine is just matmul + reduce
            nc.vector.memset(cnt[:, NCOL - 1:NCOL],
                             float(C * sum(_ACT_N)))
            scrA = sm.tile([P, max(_DVE_N), C], mybir.dt.uint8, tag="scrA")
            scrB = sm.tile([P, max(_ACT_N), C], mybir.dt.bfloat16,
                           tag="scrB")
            biasT = sm.tile([P, 1], F32, tag="biasT")
            nc.gpsimd.memset(biasT, -MIN_CONF)

            col_dve, col_act = 0, NDVE
            for b, lo, hi, dve_n, _ in CHUNKS:
                if dve_n > 0:
                    nc.vector.tensor_scalar(
                        out=scrA[:, 0:dve_n], in0=ptw[:, b, lo:lo + dve_n],
                        scalar1=MIN_CONF, scalar2=None,
                        op0=mybir.AluOpType.is_ge, op1=mybir.AluOpType.add,
                        accum_out=cnt[:, col_dve:col_dve + 1],
                    )
                    col_dve += 1
                act_n = hi - lo - dve_n
                if act_n > 0:
                    nc.scalar.activation(
                        scrB[:, 0:act_n], ptw[:, b, lo + dve_n:hi],
                        mybir.ActivationFunctionType.Sign,
                        bias=biasT[:, 0:1],
                        accum_out=cnt[:, col_act:col_act + 1],
                    )
                    col_act += 1

            ones = sm.tile([P, 1], F32, tag="ones")
            nc.vector.memset(ones, 1.0)
            # g = 2*sum(DVE counts) + sum(ACT sign sums) + #ACT-elems
            #   = 2 * (total elements >= MIN_CONF)   (exact in f32)
            nc.vector.tensor_scalar_mul(cnt[:, 0:NDVE], cnt[:, 0:NDVE], 2.0)
            csum = pp.tile([1, NCOL], F32, tag="csum")
            nc.tensor.matmul(csum, ones, cnt, start=True, stop=True)
            gi = sm.tile([1, 1], I32, tag="gi")
            with nc.allow_low_precision(
                    reason="exact small-int sum, int32 output"):
                nc.vector.reduce_sum(gi, csum, axis=mybir.AxisListType.X)

            gv = nc.values_load(gi[0:1, 0:1], min_val=0,
                                max_val=2 * BPC * N * C,
                                skip_runtime_bounds_check=True)

            # ---------------- guarded cold path ----------------
            if not NOGUARD:
              with tc.If(gv >= 1):
                NT = BPC * NP
                crev = sm.tile([P, NT, C], F32, tag="crev")
                nc.gpsimd.iota(crev, pattern=[[0, NT], [-1, C]], base=C - 1,
                               channel_multiplier=0,
                               allow_small_or_imprecise_dtypes=True)
                negs = sm.tile([P, NT], F32, tag="negs")
                nc.gpsimd.memset(negs, NEG)
                mr = sm.tile([P, 8], F32, tag="mr")
                nc.gpsimd.memset(mr, NEG)

                sc_w = sm.tile([P, BPC, NP], F32, tag="sc")
                ob_w = sm.tile([P, BPC, NP, 4], F32, tag="ob")
                ar_w = sm.tile([P, BPC, NP], F32, tag="ar")
                cat_w = sm.tile([P, BPC, NP, 6], F32, tag="cat")
                state = {
                    "negs": negs,
                    "mr": mr,
                    "sc": sc_w,
                    "ob": ob_w,
                    "ar": ar_w,
                    "cat": cat_w,
                }
                scw = sm.tile([P, BPC, NP], F32, tag="scores")

                rt = sm.tile([P, BPC, NP, 4], F32, tag="rois")
                dt_ = big.tile([P, BPC, NP, C, 4], F32, tag="deltas")
                for img in range(BPC):
                    nc.sync.dma_start(
                        out=rt[:, img],
                        in_=rois_t[img].rearrange("(p n) k -> p n k", p=P))
                    dsrc = deltas_t[img].rearrange("(p n) c k -> p n c k", p=P)
                    for s in range(8):
                        sl = slice(16 * s, 16 * s + 16)
                        nc.sync.dma_start(out=dt_[sl, img], in_=dsrc[sl])
                _refine_twin(nc, tc, sm, ptw, scw, rt, dt_, crev, state)
                for img in range(BPC):
                    _nms_image(nc, tc, sm, img, det, state)
                if DEBUG:
                    dbgs2 = sm.tile([1, 4], F32, tag="dbgs2")
                    nc.vector.tensor_copy(dbgs2[0:1, 0:2], det[0][0:1, 0:2])
                    nc.vector.tensor_copy(dbgs2[0:1, 2:4], det[1][0:1, 0:2])
                    nc.sync.dma_start(out=dbg_t[0:1, 12:16], in_=dbgs2)
                for img in range(BPC):
                    fap = out_t[img].rearrange("k s -> (k s)").unsqueeze(0)
                    nc.sync.dma_start(out=fap, in_=det[img][0:1])
    nc.compile()
    return nc


LAST_RESULTS = None  # BassKernelResults of the most recent kernel() call


def kernel(rois, probs, deltas):
    global LAST_RESULTS
    from concourse import bass_utils

    nc = build_nc()
    in_maps = []
    for c in range(NCORES):
        sl = slice(c * BPC, (c + 1) * BPC)
        in_maps.append({
            "rois": np.ascontiguousarray(rois[sl], dtype=np.float32),
            "probs": np.ascontiguousarray(probs[sl], dtype=np.float32),
            "deltas": np.ascontiguousarray(deltas[sl], dtype=np.float32),
        })
    res = bass_utils.run_bass_kernel_spmd(nc, in_maps, core_ids=list(range(NCORES)))
    LAST_RESULTS = res
    return np.concatenate([r["out"] for r in res.results], axis=0)


if __name__ == "__main__":
    rng = np.random.default_rng(0)
    out = kernel(
        rng.random((B, N, 4), np.float32),
        rng.random((B, N, C), np.float32),
        rng.standard_normal((B, N, C, 4)).astype(np.float32),
    )
    print(out.shape, np.abs(out).max())


# revision 25
# speedup vs baseline: 1.1508x; 1.0315x over previous
"""Trainium2 Bass kernel for DetectionLayer (refine + per-class NMS).

Contract: kernel(rois, probs, deltas) with FULL inputs
  rois   [16, 4096, 4]   f32
  probs  [16, 4096, 81]  f32
  deltas [16, 4096, 81, 4] f32
returns [16, 100, 6] f32 detections, matching the jax reference.

Sharding: pure data parallel - 2 images per core across 8 NeuronCores.

Fast path (always executed, ~22us):
  - probs (2.65MB/core) DMA'd in 4 chunks on both HWDGE queues (SP + Act)
    so descriptor generation overlaps; rings run at the ~360GB/s HBM
    roofline.
  - Confidence gate: per chunk, DVE counts elements >= 0.7 (is_ge +
    accumulator) and the ACT engine sign-sums (sign(p - 0.7) + accum), so
    the reduction is split across two engines and pipelines with the DMA.
    A PE ones-matmul folds per-partition counts; one int32 reduce yields
    g = 2 * #elements >= min_confidence.
  - The all-zeros output is DMA'd to HBM up front (hidden under the probs
    window), so when g == 0 the kernel ends right after the branch.
Cold path (tc.If, only when g > 0): rois + deltas (10.6MB) load, twin-image
argmax-class delta select + box refine, then a fixed 100-iteration
per-class NMS (class-offset trick) per image; real detections overwrite
the zeros in HBM. Verified bit-close vs the jax reference (rel err 7e-10)
on sharpened-probs inputs.
"""

import os as _os

import numpy as np

import concourse.bacc as bacc
import concourse.bass as bass
import concourse.bass_isa as bass_isa
import concourse.mybir as mybir
from concourse.tile import TileContext

B = 16              # full batch
NCORES = 8
BPC = B // NCORES   # images per core
N = 4096            # rois per image
C = 81              # classes
K = 100             # detection_max_instances
P = 128             # SBUF partitions
NP = N // P         # rois per partition per image (32)
NEG = -1e9
MIN_CONF = 0.7
NMS_T = 0.3
F32 = mybir.dt.float32
I32 = mybir.dt.int32

# gate split: DVE handles rois [0, NA), ACT handles [NA, NP) of each image
DEBUG = _os.environ.get("DETK_DEBUG", "0") == "1"
NOGUARD = _os.environ.get("DETK_NOGUARD", "0") == "1"
# probs DMA chunks in issue order: (img, lo, hi, dve_rois, queue)
# queue 0 = SP HWDGE (rings prioritize it), 1 = Act HWDGE (lands last).
# Per chunk, DVE counts rois [lo, lo+dve) via is_ge+sum (coeff 2) and the
# ACT engine does [lo+dve, hi) via sign+sum (coeff 1). Chunks are sized so
# the last-landing chunks carry little gate work.
CHUNKS = [
    (0, 0, 32, 17, 0),
    (1, 0, 18, 13, 0),
    (1, 18, 26, 8, 0),
    (1, 26, 32, 0, 1),
]
if _os.environ.get("DETK_CHUNKS"):
    import json as _json
    CHUNKS = [tuple(c) for c in _json.loads(_os.environ["DETK_CHUNKS"])]


def _chunk_meta(chunks):
    dve = [d for (_, _, _, d, _) in chunks if d > 0]
    act = [hi - lo - d for (_, lo, hi, d, _) in chunks if hi - lo - d > 0]
    return dve, act


_DVE_N, _ACT_N = _chunk_meta(CHUNKS)


def _refine_twin(nc, tc, sm, ptw, scw, rt, dt_, crev, state):
    """Cold path, both images at once: select argmax-class delta, refine
    boxes, build NMS state. Ops run on the flattened [P, BPC*NP] roi axis;
    the NMS loops later use per-image [:, img] slices of the same tiles."""
    NT = BPC * NP
    pt = ptw.rearrange("p b n c -> p (b n) c")          # [P, NT, C]
    scores = scw.rearrange("p b n -> p (b n)")          # [P, NT]
    rtf = rt.rearrange("p b n k -> p (b n) k")          # [P, NT, 4]

    nc.vector.reduce_max(scores, pt, axis=mybir.AxisListType.X)
    ge = sm.tile([P, NT], F32, tag="ge")
    nc.vector.tensor_single_scalar(ge, scores, MIN_CONF,
                                   op=mybir.AluOpType.is_ge)

    # one-hot mask of argmax class: M = (probs == score), in place over probs
    m = pt
    nc.vector.tensor_tensor(
        m, pt, scores.unsqueeze(2).to_broadcast([P, NT, C]),
        op=mybir.AluOpType.is_equal,
    )

    # select argmax-class delta: deltas *= M (bcast over k), sum over c
    d_perm = dt_.rearrange("p b n c k -> p (b n) k c")
    nc.vector.tensor_tensor(
        d_perm, d_perm, m.unsqueeze(2).to_broadcast([P, NT, 4, C]),
        op=mybir.AluOpType.mult,
    )
    dsel = sm.tile([P, NT, 4], F32, tag="dsel")
    nc.vector.reduce_sum(dsel, d_perm, axis=mybir.AxisListType.X)

    # class id = 80 - max((80-c) * M)  (ties -> smallest c, like argmax)
    nc.vector.tensor_tensor(m, m, crev, op=mybir.AluOpType.mult)
    cid = sm.tile([P, NT], F32, tag="cid")
    nc.vector.reduce_max(cid, m, axis=mybir.AxisListType.X)
    nc.vector.tensor_scalar(
        out=cid, in0=cid, scalar1=-1.0, scalar2=float(C - 1),
        op0=mybir.AluOpType.mult, op1=mybir.AluOpType.add,
    )

    # bbox_std scaling (match reference op order exactly)
    nc.vector.tensor_scalar_mul(dsel[:, :, 0:2], dsel[:, :, 0:2], 0.1)
    nc.vector.tensor_scalar_mul(dsel[:, :, 2:4], dsel[:, :, 2:4], 0.2)

    # ---- apply deltas + clip (mirrors _apply_deltas fp32 op order) ----
    h = sm.tile([P, NT], F32, tag="h")
    w = sm.tile([P, NT], F32, tag="w")
    nc.vector.tensor_sub(h, rtf[:, :, 2], rtf[:, :, 0])
    nc.vector.tensor_sub(w, rtf[:, :, 3], rtf[:, :, 1])
    t1 = sm.tile([P, NT], F32, tag="t1")
    t2 = sm.tile([P, NT], F32, tag="t2")
    cy = sm.tile([P, NT], F32, tag="cy")
    cx = sm.tile([P, NT], F32, tag="cx")
    nc.vector.tensor_scalar_mul(t1, h, 0.5)
    nc.vector.tensor_add(t2, rtf[:, :, 0], t1)
    nc.vector.tensor_mul(t1, dsel[:, :, 0], h)
    nc.vector.tensor_add(cy, t2, t1)
    nc.vector.tensor_scalar_mul(t1, w, 0.5)
    nc.vector.tensor_add(t2, rtf[:, :, 1], t1)
    nc.vector.tensor_mul(t1, dsel[:, :, 1], w)
    nc.vector.tensor_add(cx, t2, t1)
    e = sm.tile([P, NT], F32, tag="e")
    nc.scalar.activation(e, dsel[:, :, 2], mybir.ActivationFunctionType.Exp)
    nc.vector.tensor_mul(h, h, e)
    nc.scalar.activation(e, dsel[:, :, 3], mybir.ActivationFunctionType.Exp)
    nc.vector.tensor_mul(w, w, e)

    ref = sm.tile([P, NT, 4], F32, tag="ref")
    nc.vector.tensor_scalar_mul(t1, h, 0.5)
    nc.vector.tensor_sub(ref[:, :, 0], cy, t1)
    nc.vector.tensor_add(ref[:, :, 2], cy, t1)
    nc.vector.tensor_scalar_mul(t2, w, 0.5)
    nc.vector.tensor_sub(ref[:, :, 1], cx, t2)
    nc.vector.tensor_add(ref[:, :, 3], cx, t2)
    nc.vector.tensor_scalar(
        out=ref, in0=ref, scalar1=0.0, scalar2=1.0,
        op0=mybir.AluOpType.max, op1=mybir.AluOpType.min,
    )

    # ---- NMS state ----
    sc = state["sc"].rearrange("p b n -> p (b n)")
    ob = state["ob"].rearrange("p b n k -> p (b n) k")
    ar = state["ar"].rearrange("p b n -> p (b n)")
    cat = state["cat"].rearrange("p b n k -> p (b n) k")
    negs = state["negs"]

    vf = sm.tile([P, NT], F32, tag="vf")
    nc.vector.tensor_single_scalar(vf, cid, 0.5, op=mybir.AluOpType.is_ge)
    v = sm.tile([P, NT], mybir.dt.uint8, tag="v")
    nc.vector.tensor_mul(v, vf, ge)
    nc.vector.tensor_copy(sc, negs)
    nc.vector.copy_predicated(sc, v, scores)

    nc.vector.scalar_tensor_tensor(
        out=ob, in0=cid.unsqueeze(2).to_broadcast([P, NT, 4]), scalar=2.0,
        in1=ref, op0=mybir.AluOpType.mult, op1=mybir.AluOpType.add,
    )
    ar2 = sm.tile([P, NT, 2], F32, tag="ar2")
    nc.vector.tensor_sub(ar2, ob[:, :, 2:4], ob[:, :, 0:2])
    nc.vector.tensor_mul(ar, ar2[:, :, 0], ar2[:, :, 1])
    nc.vector.tensor_copy(cat[:, :, 0:4], ref)
    nc.vector.tensor_copy(cat[:, :, 4], cid)
    nc.vector.tensor_copy(cat[:, :, 5], scores)


def _nms_image(nc, tc, sm, img, det, state):
    """Cold path per image: fixed K-iteration NMS; rows past exhaustion are
    written as exact zeros (gm == NEG gate)."""
    sc = state["sc"][:, img]
    ob = state["ob"][:, img]
    ar = state["ar"][:, img]
    cat = state["cat"][:, img]
    negs = state["negs"][:, 0:NP]
    mr = state["mr"]

    with tc.For_i(0, K, name=f"nms{img}") as i:
        pm = sm.tile([P, 1], F32, tag=f"pm{img}")
        nc.vector.reduce_max(pm, sc, axis=mybir.AxisListType.X)
        gm = sm.tile([P, 1], F32, tag=f"gm{img}")
        nc.gpsimd.partition_all_reduce(gm, pm, channels=P,
                                       reduce_op=bass_isa.ReduceOp.max)
        msk = sm.tile([P, NP], F32, tag=f"msk{img}")
        nc.vector.tensor_tensor(msk, sc, gm.to_broadcast([P, NP]),
                                op=mybir.AluOpType.is_equal)
        mb6 = sm.tile([P, NP, 6], F32, tag=f"mb6{img}")
        nc.vector.tensor_tensor(
            mb6, cat, msk.unsqueeze(2).to_broadcast([P, NP, 6]),
            op=mybir.AluOpType.mult,
        )
        r6p = sm.tile([P, 6], F32, tag=f"r6p{img}")
        nc.vector.reduce_sum(r6p, mb6.rearrange("p n k -> p k n"),
                             axis=mybir.AxisListType.X)
        r6 = sm.tile([P, 6], F32, tag=f"r6{img}")
        nc.gpsimd.partition_all_reduce(r6, r6p, channels=P,
                                       reduce_op=bass_isa.ReduceOp.add)
        okm = sm.tile([P, 1], F32, tag=f"okm{img}")
        nc.vector.tensor_single_scalar(okm, gm, NEG * 0.5,
                                       op=mybir.AluOpType.is_gt)
        nc.vector.tensor_mul(r6, r6, okm.to_broadcast([P, 6]))
        nc.vector.tensor_copy(det[img][0:1, bass.ds(i * 6, 6)],
                              r6[0:1, :])

        sb = sm.tile([P, 4], F32, tag=f"sb{img}")
        nc.vector.scalar_tensor_tensor(
            out=sb, in0=r6[:, 4:5].to_broadcast([P, 4]), scalar=2.0,
            in1=r6[:, 0:4], op0=mybir.AluOpType.mult, op1=mybir.AluOpType.add,
        )
        mx = sm.tile([P, NP, 2], F32, tag=f"mx{img}")
        nc.vector.tensor_tensor(
            mx, ob[:, :, 0:2], sb[:, 0:2].unsqueeze(1).to_broadcast([P, NP, 2]),
            op=mybir.AluOpType.max,
        )
        mn = sm.tile([P, NP, 2], F32, tag=f"mn{img}")
        nc.vector.tensor_tensor(
            mn, ob[:, :, 2:4], sb[:, 2:4].unsqueeze(1).to_broadcast([P, NP, 2]),
            op=mybir.AluOpType.min,
        )
        nc.vector.tensor_sub(mn, mn, mx)
        nc.vector.tensor_scalar_max(mn, mn, 0.0)
        inter = sm.tile([P, NP], F32, tag=f"inter{img}")
        nc.vector.tensor_mul(inter, mn[:, :, 0], mn[:, :, 1])
        aa2 = sm.tile([P, 2], F32, tag=f"aa2{img}")
        nc.vector.tensor_sub(aa2, sb[:, 2:4], sb[:, 0:2])
        aa = sm.tile([P, 1], F32, tag=f"aa{img}")
        nc.vector.tensor_mul(aa, aa2[:, 0:1], aa2[:, 1:2])
        u = sm.tile([P, NP], F32, tag=f"u{img}")
        nc.vector.scalar_tensor_tensor(
            out=u, in0=ar, scalar=aa[:, 0:1], in1=inter,
            op0=mybir.AluOpType.add, op1=mybir.AluOpType.subtract,
        )
        sup = sm.tile([P, NP], mybir.dt.uint8, tag=f"sup{img}")
        nc.vector.scalar_tensor_tensor(
            out=sup, in0=u, scalar=NMS_T, in1=inter,
            op0=mybir.AluOpType.mult, op1=mybir.AluOpType.is_lt,
        )
        nc.vector.copy_predicated(sc, sup, negs)
        nc.vector.tensor_copy(mr[:, 0:1], gm)
        nc.vector.match_replace(out=sc, in_to_replace=mr, in_values=sc,
                                imm_value=NEG)


def build_nc():
    nc = bacc.Bacc("TRN2", target_bir_lowering=False)
    rois_t = nc.dram_tensor("rois", [BPC, N, 4], F32, kind="ExternalInput")
    probs_t = nc.dram_tensor("probs", [BPC, N, C], F32, kind="ExternalInput")
    deltas_t = nc.dram_tensor("deltas", [BPC, N, C, 4], F32, kind="ExternalInput")
    out_t = nc.dram_tensor("out", [BPC, K, 6], F32, kind="ExternalOutput")
    dbg_t = None
    if DEBUG:
        dbg_t = nc.dram_tensor("dbg", [1, 16], F32, kind="ExternalOutput")

    with TileContext(nc) as tc:
        with (
            tc.tile_pool(name="big", bufs=1) as big,
            tc.tile_pool(name="small", bufs=1) as sm,
            tc.tile_pool(name="psum", bufs=1, space="PSUM") as pp,
        ):
            # ---------------- fast path ----------------
            # probs for both images in one twin tile, DMA'd per CHUNKS on
            # two HWDGE queues (SP + Act) for parallel descriptor gen
            ptw = big.tile([P, BPC, NP, C], F32, tag="probs")
            psrc = [probs_t[b].rearrange("(p n) c -> p n c", p=P)
                    for b in range(BPC)]
            for b, lo, hi, _, q in CHUNKS:
                eng = nc.sync if q == 0 else nc.scalar
                eng.dma_start(out=ptw[:, b, lo:hi], in_=psrc[b][:, lo:hi])

            det0 = sm.tile([1, K * 6], F32, tag="det0")
            det1 = sm.tile([1, K * 6], F32, tag="det1")
            det = [det0, det1]
            nc.vector.memset(det0, 0.0)
            nc.gpsimd.memset(det1, 0.0)

            # zeros out-DMA up front; real detections overwrite in the guard
            out_aps = []
            for img in range(BPC):
                ap = out_t[img].rearrange("k s -> (k s)").unsqueeze(0)
                out_aps.append(ap)
                nc.sync.dma_start(out=ap, in_=det[img][0:1])

            # element count >= MIN_CONF: DVE is_ge+sum (coeff 2) then
            # ACT sign+sum (coeff 1); cnt cols = DVE chunks then ACT chunks
            NDVE = len(_DVE_N)
            NCOL = NDVE + len(_ACT_N) + 1
            cnt = sm.tile([P, NCOL], F32, tag="cnt")
            # last col pre-set to the per-partition sign-sum offset so the
            # final combine is just matmul + reduce
            nc.vector.memset(cnt[:, NCOL - 1:NCOL],
                             float(C * sum(_ACT_N)))
            scrA = sm.tile([P, max(_DVE_N), C], mybir.dt.uint8, tag="scrA")
            scrB = sm.tile([P, max(_ACT_N), C], mybir.dt.bfloat16,
                           tag="scrB")
            biasT = sm.tile([P, 1], F32, tag="biasT")
            nc.gpsimd.memset(biasT, -MIN_CONF)

            col_dve, col_act = 0, NDVE
            for b, lo, hi, dve_n, _ in CHUNKS:
                if dve_n > 0:
                    nc.vector.tensor_scalar(
                        out=scrA[:, 0:dve_n], in0=ptw[:, b, lo:lo + dve_n],
                        scalar1=MIN_CONF, scalar2=None,
                        op0=mybir.AluOpType.is_ge, op1=mybir.AluOpType.add,
                        accum_out=cnt[:, col_dve:col_dve + 1],
                    )
                    col_dve += 1
                act_n = hi - lo - dve_n
                if act_n > 0:
                    nc.scalar.activation(
                        scrB[:, 0:act_n], ptw[:, b, lo + dve_n:hi],
                        mybir.ActivationFunctionType.Sign,
                        bias=biasT[:, 0:1],
                        accum_out=cnt[:, col_act:col_act + 1],
                    )
                    col_act += 1

            ones = sm.tile([P, 1], F32, tag="ones")
            nc.vector.memset(ones, 1.0)
            # g = 2*sum(DVE counts) + sum(ACT sign sums) + #ACT-elems
            #   = 2 * (total elements >= MIN_CONF)   (exact in f32)
            nc.vector.tensor_scalar_mul(cnt[:, 0:NDVE], cnt[:, 0:NDVE], 2.0)
            csum = pp.tile([1, NCOL], F32, tag="csum")
            nc.tensor.matmul(csum, ones, cnt, start=True, stop=True)
            gi = sm.tile([1, 1], I32, tag="gi")
            with nc.allow_low_precision(
                    reason="exact small-int sum, int32 output"):
                nc.vector.reduce_sum(gi, csum, axis=mybir.AxisListType.X)

            gv = nc.values_load(gi[0:1, 0:1], min_val=0,
                                max_val=2 * BPC * N * C,
                                skip_runtime_bounds_check=True)

            # ---------------- guarded cold path ----------------
            if not NOGUARD:
              with tc.If(gv >= 1):
                NT = BPC * NP
                crev = sm.tile([P, NT, C], F32, tag="crev")
                nc.gpsimd.iota(crev, pattern=[[0, NT], [-1, C]], base=C - 1,
                               channel_multiplier=0,
                               allow_small_or_imprecise_dtypes=True)
                negs = sm.tile([P, NT], F32, tag="negs")
                nc.gpsimd.memset(negs, NEG)
                mr = sm.tile([P, 8], F32, tag="mr")
                nc.gpsimd.memset(mr, NEG)

                sc_w = sm.tile([P, BPC, NP], F32, tag="sc")
                ob_w = sm.tile([P, BPC, NP, 4], F32, tag="ob")
                ar_w = sm.tile([P, BPC, NP], F32, tag="ar")
                cat_w = sm.tile([P, BPC, NP, 6], F32, tag="cat")
                state = {
                    "negs": negs,
                    "mr": mr,
                    "sc": sc_w,
                    "ob": ob_w,
                    "ar": ar_w,
                    "cat": cat_w,
                }
                scw = sm.tile([P, BPC, NP], F32, tag="scores")

                rt = sm.tile([P, BPC, NP, 4], F32, tag="rois")
                dt_ = big.tile([P, BPC, NP, C, 4], F32, tag="deltas")
                for img in range(BPC):
                    nc.sync.dma_start(
                        out=rt[:, img],
                        in_=rois_t[img].rearrange("(p n) k -> p n k", p=P))
                    dsrc = deltas_t[img].rearrange("(p n) c k -> p n c k", p=P)
                    for s in range(8):
                        sl = slice(16 * s, 16 * s + 16)
                        nc.sync.dma_start(out=dt_[sl, img], in_=dsrc[sl])
                _refine_twin(nc, tc, sm, ptw, scw, rt, dt_, crev, state)
                for img in range(BPC):
                    _nms_image(nc, tc, sm, img, det, state)
                if DEBUG:
                    dbgs2 = sm.tile([1, 4], F32, tag="dbgs2")
                    nc.vector.tensor_copy(dbgs2[0:1, 0:2], det[0][0:1, 0:2])
                    nc.vector.tensor_copy(dbgs2[0:1, 2:4], det[1][0:1, 0:2])
                    nc.sync.dma_start(out=dbg_t[0:1, 12:16], in_=dbgs2)
                for img in range(BPC):
                    fap = out_t[img].rearrange("k s -> (k s)").unsqueeze(0)
                    nc.sync.dma_start(out=fap, in_=det[img][0:1])
    nc.compile()
    return nc


LAST_RESULTS = None  # BassKernelResults of the most recent kernel() call


def kernel(rois, probs, deltas):
    global LAST_RESULTS
    from concourse import bass_utils

    nc = build_nc()
    in_maps = []
    for c in range(NCORES):
        sl = slice(c * BPC, (c + 1) * BPC)
        in_maps.append({
            "rois": np.ascontiguousarray(rois[sl], dtype=np.float32),
            "probs": np.ascontiguousarray(probs[sl], dtype=np.float32),
            "deltas": np.ascontiguousarray(deltas[sl], dtype=np.float32),
        })
    res = bass_utils.run_bass_kernel_spmd(nc, in_maps, core_ids=list(range(NCORES)))
    LAST_RESULTS = res
    return np.concatenate([r["out"] for r in res.results], axis=0)


if __name__ == "__main__":
    rng = np.random.default_rng(0)
    out = kernel(
        rng.random((B, N, 4), np.float32),
        rng.random((B, N, C), np.float32),
        rng.standard_normal((B, N, C, 4)).astype(np.float32),
    )
    print(out.shape, np.abs(out).max())


# revision 26
# speedup vs baseline: 1.1562x; 1.0047x over previous
"""Trainium2 Bass kernel for DetectionLayer (refine + per-class NMS).

Contract: kernel(rois, probs, deltas) with FULL inputs
  rois   [16, 4096, 4]   f32
  probs  [16, 4096, 81]  f32
  deltas [16, 4096, 81, 4] f32
returns [16, 100, 6] f32 detections, matching the jax reference.

Sharding: pure data parallel - 2 images per core across 8 NeuronCores.

Fast path (always executed, ~22us):
  - probs (2.65MB/core) DMA'd in 4 chunks on both HWDGE queues (SP + Act)
    so descriptor generation overlaps; rings run at the ~360GB/s HBM
    roofline.
  - Confidence gate: per chunk, DVE counts elements >= 0.7 (is_ge +
    accumulator) and the ACT engine sign-sums (sign(p - 0.7) + accum), so
    the reduction is split across two engines and pipelines with the DMA.
    A PE ones-matmul folds per-partition counts; one int32 reduce yields
    g = 2 * #elements >= min_confidence.
  - The all-zeros output is DMA'd to HBM up front (hidden under the probs
    window), so when g == 0 the kernel ends right after the branch.
Cold path (tc.If, only when g > 0): rois + deltas (10.6MB) load, twin-image
argmax-class delta select + box refine, then a fixed 100-iteration
per-class NMS (class-offset trick) per image; real detections overwrite
the zeros in HBM. Verified bit-close vs the jax reference (rel err 7e-10)
on sharpened-probs inputs.
"""

import os as _os

import numpy as np

import concourse.bacc as bacc
import concourse.bass as bass
import concourse.bass_isa as bass_isa
import concourse.mybir as mybir
from concourse.tile import TileContext

B = 16              # full batch
NCORES = 8
BPC = B // NCORES   # images per core
N = 4096            # rois per image
C = 81              # classes
K = 100             # detection_max_instances
P = 128             # SBUF partitions
NP = N // P         # rois per partition per image (32)
NEG = -1e9
MIN_CONF = 0.7
NMS_T = 0.3
F32 = mybir.dt.float32
I32 = mybir.dt.int32

# gate split: DVE handles rois [0, NA), ACT handles [NA, NP) of each image
DEBUG = _os.environ.get("DETK_DEBUG", "0") == "1"
NOGUARD = _os.environ.get("DETK_NOGUARD", "0") == "1"
# probs DMA chunks in issue order: (img, lo, hi, dve_rois, queue)
# queue 0 = SP HWDGE (rings prioritize it), 1 = Act HWDGE (lands last).
# Per chunk, DVE counts rois [lo, lo+dve) via is_ge+sum (coeff 2) and the
# ACT engine does [lo+dve, hi) via sign+sum (coeff 1). Chunks are sized so
# the last-landing chunks carry little gate work.
CHUNKS = [
    (0, 0, 32, 17, 0),
    (1, 0, 18, 13, 0),
    (1, 18, 26, 8, 0),
    (1, 26, 32, 0, 1),
]
if _os.environ.get("DETK_CHUNKS"):
    import json as _json
    CHUNKS = [tuple(c) for c in _json.loads(_os.environ["DETK_CHUNKS"])]


def _chunk_meta(chunks):
    dve = [d for (_, _, _, d, _) in chunks if d > 0]
    act = [hi - lo - d for (_, lo, hi, d, _) in chunks if hi - lo - d > 0]
    return dve, act


_DVE_N, _ACT_N = _chunk_meta(CHUNKS)


def _refine_twin(nc, tc, sm, ptw, scw, rt, dt_, crev, state):
    """Cold path, both images at once: select argmax-class delta, refine
    boxes, build NMS state. Ops run on the flattened [P, BPC*NP] roi axis;
    the NMS loops later use per-image [:, img] slices of the same tiles."""
    NT = BPC * NP
    pt = ptw.rearrange("p b n c -> p (b n) c")          # [P, NT, C]
    scores = scw.rearrange("p b n -> p (b n)")          # [P, NT]
    rtf = rt.rearrange("p b n k -> p (b n) k")          # [P, NT, 4]

    nc.vector.reduce_max(scores, pt, axis=mybir.AxisListType.X)
    ge = sm.tile([P, NT], F32, tag="ge")
    nc.vector.tensor_single_scalar(ge, scores, MIN_CONF,
                                   op=mybir.AluOpType.is_ge)

    # one-hot mask of argmax class: M = (probs == score), in place over probs
    m = pt
    nc.vector.tensor_tensor(
        m, pt, scores.unsqueeze(2).to_broadcast([P, NT, C]),
        op=mybir.AluOpType.is_equal,
    )

    # select argmax-class delta: deltas *= M (bcast over k), sum over c
    d_perm = dt_.rearrange("p b n c k -> p (b n) k c")
    nc.vector.tensor_tensor(
        d_perm, d_perm, m.unsqueeze(2).to_broadcast([P, NT, 4, C]),
        op=mybir.AluOpType.mult,
    )
    dsel = sm.tile([P, NT, 4], F32, tag="dsel")
    nc.vector.reduce_sum(dsel, d_perm, axis=mybir.AxisListType.X)

    # class id = 80 - max((80-c) * M)  (ties -> smallest c, like argmax)
    nc.vector.tensor_tensor(m, m, crev, op=mybir.AluOpType.mult)
    cid = sm.tile([P, NT], F32, tag="cid")
    nc.vector.reduce_max(cid, m, axis=mybir.AxisListType.X)
    nc.vector.tensor_scalar(
        out=cid, in0=cid, scalar1=-1.0, scalar2=float(C - 1),
        op0=mybir.AluOpType.mult, op1=mybir.AluOpType.add,
    )

    # bbox_std scaling (match reference op order exactly)
    nc.vector.tensor_scalar_mul(dsel[:, :, 0:2], dsel[:, :, 0:2], 0.1)
    nc.vector.tensor_scalar_mul(dsel[:, :, 2:4], dsel[:, :, 2:4], 0.2)

    # ---- apply deltas + clip (mirrors _apply_deltas fp32 op order) ----
    h = sm.tile([P, NT], F32, tag="h")
    w = sm.tile([P, NT], F32, tag="w")
    nc.vector.tensor_sub(h, rtf[:, :, 2], rtf[:, :, 0])
    nc.vector.tensor_sub(w, rtf[:, :, 3], rtf[:, :, 1])
    t1 = sm.tile([P, NT], F32, tag="t1")
    t2 = sm.tile([P, NT], F32, tag="t2")
    cy = sm.tile([P, NT], F32, tag="cy")
    cx = sm.tile([P, NT], F32, tag="cx")
    nc.vector.tensor_scalar_mul(t1, h, 0.5)
    nc.vector.tensor_add(t2, rtf[:, :, 0], t1)
    nc.vector.tensor_mul(t1, dsel[:, :, 0], h)
    nc.vector.tensor_add(cy, t2, t1)
    nc.vector.tensor_scalar_mul(t1, w, 0.5)
    nc.vector.tensor_add(t2, rtf[:, :, 1], t1)
    nc.vector.tensor_mul(t1, dsel[:, :, 1], w)
    nc.vector.tensor_add(cx, t2, t1)
    e = sm.tile([P, NT], F32, tag="e")
    nc.scalar.activation(e, dsel[:, :, 2], mybir.ActivationFunctionType.Exp)
    nc.vector.tensor_mul(h, h, e)
    nc.scalar.activation(e, dsel[:, :, 3], mybir.ActivationFunctionType.Exp)
    nc.vector.tensor_mul(w, w, e)

    ref = sm.tile([P, NT, 4], F32, tag="ref")
    nc.vector.tensor_scalar_mul(t1, h, 0.5)
    nc.vector.tensor_sub(ref[:, :, 0], cy, t1)
    nc.vector.tensor_add(ref[:, :, 2], cy, t1)
    nc.vector.tensor_scalar_mul(t2, w, 0.5)
    nc.vector.tensor_sub(ref[:, :, 1], cx, t2)
    nc.vector.tensor_add(ref[:, :, 3], cx, t2)
    nc.vector.tensor_scalar(
        out=ref, in0=ref, scalar1=0.0, scalar2=1.0,
        op0=mybir.AluOpType.max, op1=mybir.AluOpType.min,
    )

    # ---- NMS state ----
    sc = state["sc"].rearrange("p b n -> p (b n)")
    ob = state["ob"].rearrange("p b n k -> p (b n) k")
    ar = state["ar"].rearrange("p b n -> p (b n)")
    cat = state["cat"].rearrange("p b n k -> p (b n) k")
    negs = state["negs"]

    vf = sm.tile([P, NT], F32, tag="vf")
    nc.vector.tensor_single_scalar(vf, cid, 0.5, op=mybir.AluOpType.is_ge)
    v = sm.tile([P, NT], mybir.dt.uint8, tag="v")
    nc.vector.tensor_mul(v, vf, ge)
    nc.vector.tensor_copy(sc, negs)
    nc.vector.copy_predicated(sc, v, scores)

    nc.vector.scalar_tensor_tensor(
        out=ob, in0=cid.unsqueeze(2).to_broadcast([P, NT, 4]), scalar=2.0,
        in1=ref, op0=mybir.AluOpType.mult, op1=mybir.AluOpType.add,
    )
    ar2 = sm.tile([P, NT, 2], F32, tag="ar2")
    nc.vector.tensor_sub(ar2, ob[:, :, 2:4], ob[:, :, 0:2])
    nc.vector.tensor_mul(ar, ar2[:, :, 0], ar2[:, :, 1])
    nc.vector.tensor_copy(cat[:, :, 0:4], ref)
    nc.vector.tensor_copy(cat[:, :, 4], cid)
    nc.vector.tensor_copy(cat[:, :, 5], scores)


def _nms_image(nc, tc, sm, img, det, state):
    """Cold path per image: fixed K-iteration NMS; rows past exhaustion are
    written as exact zeros (gm == NEG gate)."""
    sc = state["sc"][:, img]
    ob = state["ob"][:, img]
    ar = state["ar"][:, img]
    cat = state["cat"][:, img]
    negs = state["negs"][:, 0:NP]
    mr = state["mr"]

    with tc.For_i(0, K, name=f"nms{img}") as i:
        pm = sm.tile([P, 1], F32, tag=f"pm{img}")
        nc.vector.reduce_max(pm, sc, axis=mybir.AxisListType.X)
        gm = sm.tile([P, 1], F32, tag=f"gm{img}")
        nc.gpsimd.partition_all_reduce(gm, pm, channels=P,
                                       reduce_op=bass_isa.ReduceOp.max)
        msk = sm.tile([P, NP], F32, tag=f"msk{img}")
        nc.vector.tensor_tensor(msk, sc, gm.to_broadcast([P, NP]),
                                op=mybir.AluOpType.is_equal)
        mb6 = sm.tile([P, NP, 6], F32, tag=f"mb6{img}")
        nc.vector.tensor_tensor(
            mb6, cat, msk.unsqueeze(2).to_broadcast([P, NP, 6]),
            op=mybir.AluOpType.mult,
        )
        r6p = sm.tile([P, 6], F32, tag=f"r6p{img}")
        nc.vector.reduce_sum(r6p, mb6.rearrange("p n k -> p k n"),
                             axis=mybir.AxisListType.X)
        r6 = sm.tile([P, 6], F32, tag=f"r6{img}")
        nc.gpsimd.partition_all_reduce(r6, r6p, channels=P,
                                       reduce_op=bass_isa.ReduceOp.add)
        okm = sm.tile([P, 1], F32, tag=f"okm{img}")
        nc.vector.tensor_single_scalar(okm, gm, NEG * 0.5,
                                       op=mybir.AluOpType.is_gt)
        nc.vector.tensor_mul(r6, r6, okm.to_broadcast([P, 6]))
        nc.vector.tensor_copy(det[img][0:1, bass.ds(i * 6, 6)],
                              r6[0:1, :])

        sb = sm.tile([P, 4], F32, tag=f"sb{img}")
        nc.vector.scalar_tensor_tensor(
            out=sb, in0=r6[:, 4:5].to_broadcast([P, 4]), scalar=2.0,
            in1=r6[:, 0:4], op0=mybir.AluOpType.mult, op1=mybir.AluOpType.add,
        )
        mx = sm.tile([P, NP, 2], F32, tag=f"mx{img}")
        nc.vector.tensor_tensor(
            mx, ob[:, :, 0:2], sb[:, 0:2].unsqueeze(1).to_broadcast([P, NP, 2]),
            op=mybir.AluOpType.max,
        )
        mn = sm.tile([P, NP, 2], F32, tag=f"mn{img}")
        nc.vector.tensor_tensor(
            mn, ob[:, :, 2:4], sb[:, 2:4].unsqueeze(1).to_broadcast([P, NP, 2]),
            op=mybir.AluOpType.min,
        )
        nc.vector.tensor_sub(mn, mn, mx)
        nc.vector.tensor_scalar_max(mn, mn, 0.0)
        inter = sm.tile([P, NP], F32, tag=f"inter{img}")
        nc.vector.tensor_mul(inter, mn[:, :, 0], mn[:, :, 1])
        aa2 = sm.tile([P, 2], F32, tag=f"aa2{img}")
        nc.vector.tensor_sub(aa2, sb[:, 2:4], sb[:, 0:2])
        aa = sm.tile([P, 1], F32, tag=f"aa{img}")
        nc.vector.tensor_mul(aa, aa2[:, 0:1], aa2[:, 1:2])
        u = sm.tile([P, NP], F32, tag=f"u{img}")
        nc.vector.scalar_tensor_tensor(
            out=u, in0=ar, scalar=aa[:, 0:1], in1=inter,
            op0=mybir.AluOpType.add, op1=mybir.AluOpType.subtract,
        )
        sup = sm.tile([P, NP], mybir.dt.uint8, tag=f"sup{img}")
        nc.vector.scalar_tensor_tensor(
            out=sup, in0=u, scalar=NMS_T, in1=inter,
            op0=mybir.AluOpType.mult, op1=mybir.AluOpType.is_lt,
        )
        nc.vector.copy_predicated(sc, sup, negs)
        nc.vector.tensor_copy(mr[:, 0:1], gm)
        nc.vector.match_replace(out=sc, in_to_replace=mr, in_values=sc,
                                imm_value=NEG)


def build_nc():
    nc = bacc.Bacc("TRN2", target_bir_lowering=False)
    rois_t = nc.dram_tensor("rois", [BPC, N, 4], F32, kind="ExternalInput")
    probs_t = nc.dram_tensor("probs", [BPC, N, C], F32, kind="ExternalInput")
    deltas_t = nc.dram_tensor("deltas", [BPC, N, C, 4], F32, kind="ExternalInput")
    out_t = nc.dram_tensor("out", [BPC, K, 6], F32, kind="ExternalOutput")
    dbg_t = None
    if DEBUG:
        dbg_t = nc.dram_tensor("dbg", [1, 16], F32, kind="ExternalOutput")

    with TileContext(nc) as tc:
        with (
            tc.tile_pool(name="big", bufs=1) as big,
            tc.tile_pool(name="small", bufs=1) as sm,
            tc.tile_pool(name="psum", bufs=1, space="PSUM") as pp,
        ):
            # ---------------- fast path ----------------
            # probs for both images in one twin tile, DMA'd per CHUNKS on
            # two HWDGE queues (SP + Act) for parallel descriptor gen
            ptw = big.tile([P, BPC, NP, C], F32, tag="probs")
            psrc = [probs_t[b].rearrange("(p n) c -> p n c", p=P)
                    for b in range(BPC)]
            for b, lo, hi, _, q in CHUNKS:
                eng = nc.sync if q == 0 else nc.scalar
                eng.dma_start(out=ptw[:, b, lo:hi], in_=psrc[b][:, lo:hi])

            det0 = sm.tile([1, K * 6], F32, tag="det0")
            det1 = sm.tile([1, K * 6], F32, tag="det1")
            det = [det0, det1]
            nc.vector.memset(det0, 0.0)
            nc.gpsimd.memset(det1, 0.0)

            # zeros out-DMA up front; real detections overwrite in the guard
            out_aps = []
            for img in range(BPC):
                ap = out_t[img].rearrange("k s -> (k s)").unsqueeze(0)
                out_aps.append(ap)
                nc.sync.dma_start(out=ap, in_=det[img][0:1])

            # element count >= MIN_CONF: DVE is_ge+sum (coeff 2) then
            # ACT sign+sum (coeff 1); cnt cols = DVE chunks then ACT chunks
            NDVE = len(_DVE_N)
            NCOL = NDVE + len(_ACT_N) + 1
            cnt = sm.tile([P, NCOL], F32, tag="cnt")
            # last col pre-set to the per-partition sign-sum offset so the
            # final combine is just matmul + reduce
            nc.vector.memset(cnt[:, NCOL - 1:NCOL],
                             float(C * sum(_ACT_N)))
            scrA = sm.tile([P, max(_DVE_N), C], mybir.dt.uint8, tag="scrA")
            scrB = sm.tile([P, max(_ACT_N), C], mybir.dt.bfloat16,
                           tag="scrB")
            biasT = sm.tile([P, 1], F32, tag="biasT")
            nc.gpsimd.memset(biasT, -MIN_CONF)

            col_dve, col_act = 0, NDVE
            for b, lo, hi, dve_n, _ in CHUNKS:
                if dve_n > 0:
                    nc.vector.tensor_scalar(
                        out=scrA[:, 0:dve_n], in0=ptw[:, b, lo:lo + dve_n],
                        scalar1=MIN_CONF, scalar2=None,
                        op0=mybir.AluOpType.is_ge, op1=mybir.AluOpType.add,
                        accum_out=cnt[:, col_dve:col_dve + 1],
                    )
                    col_dve += 1
                act_n = hi - lo - dve_n
                if act_n > 0:
                    nc.scalar.activation(
                        scrB[:, 0:act_n], ptw[:, b, lo + dve_n:hi],
                        mybir.ActivationFunctionType.Sign,
                        bias=biasT[:, 0:1],
                        accum_out=cnt[:, col_act:col_act + 1],
                    )
                    col_act += 1

            ones = sm.tile([P, 1], F32, tag="ones")
            nc.vector.memset(ones, 1.0)
            # g = 2*sum(DVE counts) + sum(ACT sign sums) + #ACT-elems
            #   = 2 * (total elements >= MIN_CONF)   (exact in f32)
            nc.vector.tensor_scalar_mul(cnt[:, 0:NDVE], cnt[:, 0:NDVE], 2.0)
            csum = pp.tile([1, NCOL], F32, tag="csum")
            nc.tensor.matmul(csum, ones, cnt, start=True, stop=True)
            gi = sm.tile([1, 1], I32, tag="gi")
            with nc.allow_low_precision(
                    reason="exact small-int sum, int32 output"):
                nc.vector.reduce_sum(gi, csum, axis=mybir.AxisListType.X)

            gv = nc.values_load(gi[0:1, 0:1], min_val=0,
                                max_val=2 * BPC * N * C,
                                skip_runtime_bounds_check=True)

            # ---------------- guarded cold path ----------------
            if not NOGUARD:
              with tc.If(gv >= 1, preferred_fallthrough_block=False):
                NT = BPC * NP
                crev = sm.tile([P, NT, C], F32, tag="crev")
                nc.gpsimd.iota(crev, pattern=[[0, NT], [-1, C]], base=C - 1,
                               channel_multiplier=0,
                               allow_small_or_imprecise_dtypes=True)
                negs = sm.tile([P, NT], F32, tag="negs")
                nc.gpsimd.memset(negs, NEG)
                mr = sm.tile([P, 8], F32, tag="mr")
                nc.gpsimd.memset(mr, NEG)

                sc_w = sm.tile([P, BPC, NP], F32, tag="sc")
                ob_w = sm.tile([P, BPC, NP, 4], F32, tag="ob")
                ar_w = sm.tile([P, BPC, NP], F32, tag="ar")
                cat_w = sm.tile([P, BPC, NP, 6], F32, tag="cat")
                state = {
                    "negs": negs,
                    "mr": mr,
                    "sc": sc_w,
                    "ob": ob_w,
                    "ar": ar_w,
                    "cat": cat_w,
                }
                scw = sm.tile([P, BPC, NP], F32, tag="scores")

                rt = sm.tile([P, BPC, NP, 4], F32, tag="rois")
                dt_ = big.tile([P, BPC, NP, C, 4], F32, tag="deltas")
                for img in range(BPC):
                    nc.sync.dma_start(
                        out=rt[:, img],
                        in_=rois_t[img].rearrange("(p n) k -> p n k", p=P))
                    dsrc = deltas_t[img].rearrange("(p n) c k -> p n c k", p=P)
                    for s in range(8):
                        sl = slice(16 * s, 16 * s + 16)
                        nc.sync.dma_start(out=dt_[sl, img], in_=dsrc[sl])
                _refine_twin(nc, tc, sm, ptw, scw, rt, dt_, crev, state)
                for img in range(BPC):
                    _nms_image(nc, tc, sm, img, det, state)
                if DEBUG:
                    dbgs2 = sm.tile([1, 4], F32, tag="dbgs2")
                    nc.vector.tensor_copy(dbgs2[0:1, 0:2], det[0][0:1, 0:2])
                    nc.vector.tensor_copy(dbgs2[0:1, 2:4], det[1][0:1, 0:2])
                    nc.sync.dma_start(out=dbg_t[0:1, 12:16], in_=dbgs2)
                for img in range(BPC):
                    fap = out_t[img].rearrange("k s -> (k s)").unsqueeze(0)
                    nc.sync.dma_start(out=fap, in_=det[img][0:1])
    nc.compile()
    return nc


LAST_RESULTS = None  # BassKernelResults of the most recent kernel() call


def kernel(rois, probs, deltas):
    global LAST_RESULTS
    from concourse import bass_utils

    nc = build_nc()
    in_maps = []
    for c in range(NCORES):
        sl = slice(c * BPC, (c + 1) * BPC)
        in_maps.append({
            "rois": np.ascontiguousarray(rois[sl], dtype=np.float32),
            "probs": np.ascontiguousarray(probs[sl], dtype=np.float32),
            "deltas": np.ascontiguousarray(deltas[sl], dtype=np.float32),
        })
    res = bass_utils.run_bass_kernel_spmd(nc, in_maps, core_ids=list(range(NCORES)))
    LAST_RESULTS = res
    return np.concatenate([r["out"] for r in res.results], axis=0)


if __name__ == "__main__":
    rng = np.random.default_rng(0)
    out = kernel(
        rng.random((B, N, 4), np.float32),
        rng.random((B, N, C), np.float32),
        rng.standard_normal((B, N, C, 4)).astype(np.float32),
    )
    print(out.shape, np.abs(out).max())
